# revision 3
# baseline (speedup 1.0000x reference)
"""Trainium2 Bass kernel for nn_DigitConvolutionalModel (dense_cnn).

Model: y = relu(conv3x3(x) @ w1.T + b1) @ w2.T + b2, x: [65536, 784] f32.

Strategy (v2):
  * Conv3x3 and FC1 fuse on the host into one effective weight
    W1e = w1 @ C with shape [128, 784] (C is the sparse conv operator),
    so the device runs a pure GEMM pipeline:
    y = relu(x @ W1e.T + b1) @ w2.T + b2.
  * Pure data parallel over 8 NeuronCores: each core gets 8192 rows of x.
    No collectives; each core produces its own output shard.
  * Matmul operands travel as fp16: tf32-class accuracy for this model's
    value ranges, 1 cycle/row on the PE, half the HBM traffic for x.
    All accumulation stays fp32 in PSUM.
  * x is streamed in 16 blocks of 512 batch columns, each with its OWN
    SBUF slot (x fits in SBUF: 96 KB/partition of ~208), so no DMA ever
    waits on buffer recycling: the 16 block loads are issued back to
    back on the SP HWDGE ring and stream at fabric rate (~430 GB/s)
    end to end. Each block load is one fully contiguous 786 KB region
    (128 descriptors x 6 KB) thanks to host pre-tiling.
  * Weights / biases / the 16-row contraction tail (features 768:784 for
    the whole batch) load on the ACT HWDGE ring, in parallel with the x
    stream, so the first FC1 group never queues behind them.
  * Per 512-column block: 6 accumulating FC1 matmuls + 1 tail matmul
    into a PSUM bank (4-bank rotation), fused bias+ReLU on the vector
    engine (PSUM -> SBUF fp16), one [10, 512] FC2 matmul (3-bank
    rotation), FC2 bias on the scalar engine, then a store issued from
    the SP ring (which has finished issuing loads by then).
  * Cross-engine waits are absorbed into the PE stream with tiny dummy
    bf16 ldweights "probes"; h/o tiles get 16 dedicated slots so no
    instruction needs more than one sync wait (this walrus allows one);
    the few remaining multi-waits are split via event semaphores
    (bass_rust.generate_event_semaphores).
  * Six dummy matmuls over a zeroed scratch tile during the DMA-bound
    startup window pre-warm the PE's HAM clock gate to 2.4 GHz.
"""

import os

import numpy as np

import concourse.bass as bass
import concourse.mybir as mybir
import concourse.tile as tile
from concourse.bass import ts
from concourse.bass_utils import run_bass_kernel_spmd

H = W = 28
KH = KW = 3
CIN = H * W  # 784
HID = 128
OUT = 10
B_TOTAL = 65536
NCORES = 8
BS = B_TOTAL // NCORES  # 8192 rows per core
NB = 512  # batch columns per block (fp32 PSUM bank limit)
NBLK = BS // NB  # 16
KCH = 128
KC = 6  # full chunks (6 * 128 = 768)
KTAIL = CIN - KC * KCH  # 16

MM_MODE = os.environ.get("BASS_MM_DT", "f16")
HOST_DT = np.float16


def _build_nc():
    f32 = mybir.dt.float32
    mdt = mybir.dt.float16
    nc = bass.Bass()
    # x, host-pretiled per block: xb[bi] is one contiguous [128, 6, 512]
    # region (features 0:768 of columns bi*512:(bi+1)*512)
    xb = nc.dram_tensor("xb", [NBLK, KCH, KC, NB], mdt, kind="ExternalInput")
    # x contraction tail (features 768:784) for the whole batch
    xtl = nc.dram_tensor("xtl", [KTAIL, BS], mdt, kind="ExternalInput")
    # all fp16 weights packed into one tensor -> one DMA:
    # cols 0:768 = w1e chunks [k, c, m], rows 0:16 of cols 768:896 = the
    # 16-row w1e tail, cols 896:906 = w2t
    wpk = nc.dram_tensor("wpk", [KCH, 906], mdt, kind="ExternalInput")
    # both biases in one f32 tensor: col 0 = b1, col 1 rows 0:10 = b2
    bd = nc.dram_tensor("bd", [HID, 2], f32, kind="ExternalInput")
    yt = nc.dram_tensor("yt", [OUT, BS], f32, kind="ExternalOutput")

    with tile.TileContext(nc) as tc:
        with (
            tc.tile_pool(name="consts", bufs=1) as consts,
            tc.tile_pool(name="xin", bufs=NBLK) as xin,
            tc.tile_pool(name="hpool", bufs=NBLK) as hpool,
            tc.tile_pool(name="opool", bufs=NBLK) as opool,
            tc.tile_pool(name="ps1", bufs=4, space="PSUM") as ps1p,
            tc.tile_pool(name="ps2", bufs=3, space="PSUM") as ps2p,
        ):
            # Issue every x block load up front on the SP ring; each has
            # its own slot so none carries a wait and the ring streams
            # continuously.
            x_ts = []
            for bi in range(NBLK):
                x_t = xin.tile([KCH, KC, NB], mdt, tag="x", name=f"x_{bi}")
                nc.sync.dma_start(x_t[:], xb[bi][:])
                x_ts.append(x_t)

            # Weights / biases / tail on the ACT ring, in parallel.
            wpk_t = consts.tile([KCH, 906], mdt)
            nc.scalar.dma_start(wpk_t[:], wpk[:])
            w1_t = wpk_t[:, 0:768].rearrange("k (c m) -> k c m", c=KC)
            w1_tail = wpk_t[0:KTAIL, 768:896]
            w2_t = wpk_t[:, 896:906]
            bd_t = consts.tile([HID, 2], f32)
            nc.scalar.dma_start(bd_t[:], bd[:])
            b1_t = bd_t[:, 0:1]
            b2_t = bd_t[0:OUT, 1:2]
            x_tl = consts.tile([KTAIL, BS], mdt)
            nc.scalar.dma_start(x_tl[:], xtl[:])

            # Pre-touch the bias tiles on their consumer engines (b1 on
            # DVE, b2 on ACT) so relu / bias-add need no extra wait.
            b1_probe = consts.tile([1, 1], f32)
            nc.vector.tensor_copy(b1_probe[:], b1_t[0:1, 0:1])
            b2_probe = consts.tile([1, 1], f32)
            nc.scalar.copy(b2_probe[:], b2_t[0:1, 0:1])

            # Tiny dummy bf16 ldweights "probes" absorb cross-engine
            # waits into the PE's in-order stream ahead of each matmul
            # group (walrus: one sync wait per instruction; the loaded
            # garbage weight is irrelevant, real matmuls self-load).
            def probe(ap):
                nc.tensor.ldweights(ap[0:1, 0:1].bitcast(mybir.dt.bfloat16))

            probe(w1_t[:, 0, :])
            probe(w1_tail[:])
            probe(x_tl[:])
            probe(w2_t[:])

            # HAM warm-up: ~6 x 430 ns of dummy matmuls during the
            # startup window gets the PE past the ~3.4 us activity
            # window so real matmuls start at 2.4 GHz.
            scratch = consts.tile([KCH, NB], mdt)
            nc.gpsimd.memset(scratch[:], 0.0)
            psd = ps2p.tile([HID, NB], f32, tag="warm", bufs=1)
            for _ in range(6):
                nc.tensor.matmul(
                    psd[:], scratch[:, 0:HID], scratch[:], start=True, stop=True
                )

            for bi in range(NBLK):
                x_t = x_ts[bi]
                probe(x_t[:, 0, :])
                ps = ps1p.tile([HID, NB], f32, tag="ps")
                for c in range(KC):
                    nc.tensor.matmul(
                        ps[:],
                        w1_t[:, c, :],
                        x_t[:, c, :],
                        start=(c == 0),
                        stop=False,
                    )
                nc.tensor.matmul(
                    ps[:],
                    w1_tail[:],
                    x_tl[:, ts(bi, NB)],
                    start=False,
                    stop=True,
                )

                # relu+bias on DVE: h = max(ps + b1, 0), fp16 out
                h = hpool.tile([HID, NB], mdt, tag="h", name=f"h_{bi}")
                nc.vector.tensor_scalar(
                    h[:],
                    ps[:],
                    b1_t[:],
                    0.0,
                    mybir.AluOpType.add,
                    mybir.AluOpType.max,
                )
                probe(h[:])
                ps2 = ps2p.tile([OUT, NB], f32, tag="ps2", bufs=3)
                nc.tensor.matmul(ps2[:], w2_t[:], h[:], start=True, stop=True)

                # FC2 bias on the scalar engine, then a HWDGE store from
                # the same sequencer (keeps the SP ring free for loads)
                o = opool.tile([OUT, NB], f32, tag="o", name=f"o_{bi}")
                nc.scalar.activation(
                    o[:],
                    ps2[:],
                    mybir.ActivationFunctionType.Identity,
                    bias=b2_t[:],
                )
                nc.scalar.dma_start(yt[:, ts(bi, NB)], o[:])

    # This walrus build allows one sync-wait per instruction; Tile emits
    # multi-waits in a few places. Split them into event-semaphore
    # chains, same as bacc.compile() does.
    import bass_rust

    bass_rust.generate_event_semaphores(nc)
    return nc


def _fuse_conv_fc1(conv_w, w1):
    """W1e = w1 @ C where C is the 3x3 valid-conv operator [676, 784]."""
    cw = np.asarray(conv_w, np.float64).reshape(KH, KW)
    w1_r = np.asarray(w1, np.float64).reshape(HID, H - KH + 1, W - KW + 1)
    w1e = np.zeros((HID, H, W), np.float64)
    for a in range(KH):
        for b in range(KW):
            w1e[:, a : a + H - KH + 1, b : b + W - KW + 1] += w1_r * cw[a, b]
    return w1e.reshape(HID, CIN).astype(np.float32)


def _core_x(x_shard):
    """Pre-tile one core's x rows [BS, 784] into the device layout:
    xb [nblk, k, c, n] (features 0:768, per-block contiguous) and
    xtl [16, BS] (tail)."""
    xb = np.ascontiguousarray(
        x_shard[:, : KC * KCH]
        .reshape(NBLK, NB, KC, KCH)
        .transpose(0, 3, 2, 1)
        .astype(HOST_DT)
    )
    xtl = np.ascontiguousarray(x_shard[:, KC * KCH :].T.astype(HOST_DT))
    return xb, xtl


def _host_weights(conv_w, w1, b1, w2, b2):
    """Pack all fp16 weights into wpk [128, 906] and biases into bd."""
    w1e_t = _fuse_conv_fc1(conv_w, w1).T.astype(HOST_DT)  # [784, 128]
    w2t = np.asarray(w2, np.float32).T.astype(HOST_DT)  # [128, 10]
    wpk = np.zeros((KCH, 906), HOST_DT)
    wpk[:, 0:768] = (
        w1e_t[0 : KC * KCH].reshape(KC, KCH, HID).transpose(1, 0, 2).reshape(KCH, -1)
    )
    wpk[0:KTAIL, 768:896] = w1e_t[KC * KCH :]
    wpk[:, 896:906] = w2t
    bd = np.zeros((HID, 2), np.float32)
    bd[:, 0] = np.asarray(b1, np.float32)
    bd[0:OUT, 1] = np.asarray(b2, np.float32)
    return np.ascontiguousarray(wpk), np.ascontiguousarray(bd)


def _run(x, conv_w, w1, b1, w2, b2, trace=False):
    x = np.asarray(x, np.float32)
    wpk, bd = _host_weights(conv_w, w1, b1, w2, b2)

    nc = _build_nc()
    in_maps = []
    for c in range(NCORES):
        xb, xtl = _core_x(x[c * BS : (c + 1) * BS])
        in_maps.append({"xb": xb, "xtl": xtl, "wpk": wpk, "bd": bd})
    res = run_bass_kernel_spmd(nc, in_maps, list(range(NCORES)), trace=trace)

    y = np.empty((B_TOTAL, OUT), np.float32)
    for c, r in enumerate(res.results):
        y[c * BS : (c + 1) * BS] = r["yt"].T
    return y, res


def kernel(x, conv_w, w1, b1, w2, b2):
    y, _ = _run(x, conv_w, w1, b1, w2, b2)
    return y
